# revision 28
# baseline (speedup 1.0000x reference)
"""Trainium2 Bass kernel for nn_ActorNetwork (2-layer LSTM [T=4,H=64] + 3-layer
MLP + log_softmax over a batch of 131072 13-dim states).

Strategy: pure data parallel over 8 NeuronCores (16384 samples/core).
On-chip layout is feature-major (gate-major): gates/hidden units live on SBUF
partitions, samples on the free axis. Two 512-sample subtiles ("A" at
partitions 0:64, "B" at 64:128) are pair-packed so ACT/DVE ops run with all
128 partitions busy; gate matmuls use block-diagonal weights so one
K=128/M=128 matmul produces the gate for both subtiles. Two pairs are
emitted in lockstep so engines pipeline across them. Matmuls are bf16 with
fp32 PSUM accumulation; the cell state c and the softmax tail stay fp32.
All LSTM biases ride matmuls (ones-rows in the x blocks for layer 0, a
dedicated K=6 const matmul for layer 1) so the sigmoid over [I|F|O] is one
merged ACT op per step.
"""

import numpy as np
import ml_dtypes

import concourse.bass as bass
import concourse.mybir as mybir
from concourse.tile import TileContext
from concourse.bass_utils import run_bass_kernel_spmd
from concourse.vector_clock import ScopedClock
import concourse.tile as _tile_mod

BF16 = mybir.dt.bfloat16
F32 = mybir.dt.float32
AF = mybir.ActivationFunctionType

P = 128
FD = 512          # samples per subtile (= matmul free dim = one psum bank)
H = 64
NCORES = 8
B_TOTAL = 131072
B_CORE = B_TOTAL // NCORES          # 16384
NCHUNK = B_CORE // FD               # 32 subtiles/core
NPAIR = NCHUNK // 2                 # 16 pairs/core

# psum column region r -> PyTorch gate row range (PyTorch gate order i,f,g,o)
# regions ordered [i, f, o, g] so sigmoid covers a contiguous [I|F|O] block
GSLICE = [slice(0, 64), slice(64, 128), slice(192, 256), slice(128, 192)]
RORD = (3, 0, 1, 2)  # emit G first so tanh(G) unblocks the c-chain earliest

# ---------------------------------------------------------------------------
# walrus workaround: this toolchain rejects instructions carrying more than
# one sync wait; split excess waits onto same-engine nops inserted right
# before the offending instruction (identical engine-stream semantics).
_WAIT_LIMIT = 1


def _split_excess_waits(nc, limit=_WAIT_LIMIT):
    for f in nc.m.functions:
        for bb in f.blocks:
            snapshot = list(bb.instructions)
            out = []
            changed = False
            for inst in snapshot:
                si = getattr(inst, "sync_info", None)
                waits = list(si.on_wait) if si is not None else []
                if len(waits) > limit:
                    changed = True
                    extra, keep = waits[:-limit], waits[-limit:]
                    for w in extra:
                        b = nc.engines[inst.engine].nop(
                            nofuse=True, hint="wsplit"
                        )
                        ni = b.ins
                        cb = nc.cur_bb.bb
                        cb.instructions.remove(ni)
                        ni.sync_info = mybir.SyncInfo(
                            on_wait=[w], on_update=[]
                        )
                        out.append(ni)
                    inst.sync_info = mybir.SyncInfo(
                        on_wait=keep, on_update=list(si.on_update)
                    )
                out.append(inst)
            if changed:
                bb.instructions[:] = out


def _patched_drain_and_barrier(self, tick_clock, wait_clock):
    nc = self.nc
    drain_inst = nc.sync.drain()
    wait_clock.add_sem_waits(
        drain_inst.ins, ScopedClock({None: tick_clock.global_clock})
    )
    si = drain_inst.ins.sync_info
    waits = list(si.on_wait) if si is not None else []
    if len(waits) > _WAIT_LIMIT:
        drain_inst.ins.sync_info = mybir.SyncInfo(
            on_wait=waits[:_WAIT_LIMIT], on_update=list(si.on_update)
        )
        for k in range(_WAIT_LIMIT, len(waits), _WAIT_LIMIT):
            d2 = nc.sync.drain()
            d2.ins.sync_info = mybir.SyncInfo(
                on_wait=waits[k : k + _WAIT_LIMIT], on_update=[]
            )
    nc.all_engine_barrier()
    popped = nc._tile_sem_poison_stack.pop()
    assert popped is self._sem_poison
    nc.clear_and_free_semaphores(list(self.sems.allocated().values()))
    nc.all_engine_barrier()
    _split_excess_waits(nc)


_tile_mod.TileContext._drain_and_barrier = _patched_drain_and_barrier
# ---------------------------------------------------------------------------


class _PairCtx:
    __slots__ = ("idx", "xp6", "ft", "h0", "h1", "mlph", "c0", "c1")


def build_program(nchunk=NCHUNK):
    """Build the SPMD Bass program for one core processing nchunk*FD samples."""
    assert nchunk % 2 == 0
    npair = nchunk // 2
    ncols = nchunk * FD
    pcols = npair * FD

    nc = bass.Bass("TRN2", num_devices=NCORES)

    xq = nc.declare_dram_parameter("xq", [4, 6, pcols], BF16, isOutput=False)
    fq = nc.declare_dram_parameter("fq", [5, ncols], BF16, isOutput=False)
    wx6_d = nc.declare_dram_parameter("wx6", [6, 512], BF16, isOutput=False)
    wh0_d = nc.declare_dram_parameter("wh0d", [128, 512], BF16, isOutput=False)
    w1i_d = nc.declare_dram_parameter("w1i", [128, 512], BF16, isOutput=False)
    w1r_d = nc.declare_dram_parameter("w1r", [128, 512], BF16, isOutput=False)
    w1h_d = nc.declare_dram_parameter("w1h", [128, 30], BF16, isOutput=False)
    w1f_d = nc.declare_dram_parameter("w1f", [37, 30], BF16, isOutput=False)
    wm2_d = nc.declare_dram_parameter("wm2", [30, 10], BF16, isOutput=False)
    wm3_d = nc.declare_dram_parameter("wm3", [43, 4], BF16, isOutput=False)
    bl1_d = nc.declare_dram_parameter("bl1", [128, 4], F32, isOutput=False)
    bm1_d = nc.declare_dram_parameter("bm1", [30, 1], F32, isOutput=False)
    bm2_d = nc.declare_dram_parameter("bm2", [42, 1], F32, isOutput=False)
    no_d = nc.declare_dram_parameter("negones", [97, 4], F32, isOutput=False)
    o4_d = nc.declare_dram_parameter("ones4", [68, 1], F32, isOutput=False)
    out_d = nc.declare_dram_parameter("out", [4, ncols], F32, isOutput=True)
    warm_d = nc.declare_dram_parameter("warm", [1, 4], F32, isOutput=True)

    with TileContext(nc) as tc:
        with (
            tc.tile_pool(name="const", bufs=1) as const,
            tc.tile_pool(name="xpool", bufs=4) as xpool,
            tc.tile_pool(name="gp", bufs=3) as gp,
            tc.tile_pool(name="hp", bufs=4) as hp,
            tc.tile_pool(name="st", bufs=3) as st,
            tc.tile_pool(name="pers", bufs=npair) as pers,
            tc.tile_pool(name="pp", bufs=2, space="PSUM") as pp,
            tc.tile_pool(name="p2", bufs=2) as p2,
        ):
            # ---- constants -------------------------------------------------
            def cdma(name, dram, shape, dt=BF16):
                t = const.tile(shape, dt, name=name)
                nc.sync.dma_start(t[:], dram[:, :])
                return t

            wx6 = cdma("wx6", wx6_d, [6, 512])
            wh0 = cdma("wh0", wh0_d, [128, 512])
            w1i = cdma("w1i", w1i_d, [128, 512])
            w1r = cdma("w1r", w1r_d, [128, 512])
            w1h = cdma("w1h", w1h_d, [128, 30])
            w1f = cdma("w1f", w1f_d, [37, 30])
            wm2 = cdma("wm2", wm2_d, [30, 10])
            wm3 = cdma("wm3", wm3_d, [43, 4])
            bl1 = cdma("bl1", bl1_d, [128, 4], F32)
            bm1 = cdma("bm1", bm1_d, [30, 1], F32)
            bm2 = cdma("bm2", bm2_d, [42, 1], F32)
            nones = cdma("nones", no_d, [97, 4], F32)
            ones4 = cdma("ones4", o4_d, [68, 1], F32)
            onesbf = const.tile([1, 2048], BF16, name="onesbf")
            nc.vector.memset(onesbf[:], 1.0)

            # ---- PE warm-up: a dense burst of back-to-back matmuls so the
            # HAM clock gate reaches K=8/8 (2.4 GHz) before the real work.
            # Result is exported to a dummy output so nothing can DCE it.
            for blk in range(2):
                wps = pp.tile([128, 2048], F32, name="ps")
                for k in range(48):
                    bank = k % 4
                    nc.tensor.matmul(
                        wps[:, bank * FD : (bank + 1) * FD],
                        lhsT=wh0[:, 0:128],
                        rhs=wh0[:, 0:512],
                        start=(k < 4),
                        stop=(k >= 44),
                        tile_position=(0, 0),
                    )
                wsb = const.tile([1, 4], F32, name=f"wsb{blk}")
                nc.vector.tensor_copy(wsb[:], wps[0:1, 0:4])
                nc.sync.dma_start(warm_d[:, :], wsb[:])

            persist = []

            def open_pair(p):
                px = _PairCtx()
                px.idx = p
                pc = slice(p * FD, (p + 1) * FD)
                ca = slice(2 * p * FD, (2 * p + 1) * FD)
                cb = slice((2 * p + 1) * FD, (2 * p + 2) * FD)
                px.xp6 = []
                for t in range(4):
                    x6 = xpool.tile([6, FD], BF16, name=f"x6{t}")
                    nc.sync.dma_start(x6[:], xq[t, :, pc])
                    px.xp6.append(x6)
                ft = pers.tile([37, FD], BF16, name="ft")
                nc.sync.dma_start(ft[0:5, :], fq[:, ca])    # A feats
                nc.sync.dma_start(ft[32:37, :], fq[:, cb])  # B feats
                px.ft = ft
                px.mlph = pers.tile([P, FD], BF16, name="mlph")
                px.h0 = [hp.tile([P, FD], BF16, name=f"h0p{t}") for t in range(4)]
                px.h1 = [hp.tile([P, FD], BF16, name=f"h1p{t}") for t in range(3)]
                px.c0 = st.tile([P, FD], F32, name="c0")
                px.c1 = st.tile([P, FD], F32, name="c1")
                return px

            def emit_early(px, layer, t):
                ps = pp.tile([128, 2048], F32, name="ps")
                if layer == 0:
                    # x-projection (+bias ones-rows): depends only on the
                    # DMA'd x block and the psum slot -> runs well ahead of
                    # the recurrent chain and keeps PE busy.
                    for r in RORD:
                        nc.tensor.matmul(
                            ps[:, r * FD : (r + 1) * FD],
                            lhsT=wx6[:, 128 * r : 128 * (r + 1)],
                            rhs=px.xp6[t][:, :],
                            start=True,
                            stop=(t == 0),
                            tile_position=(0, 0),
                        )
                else:
                    if t >= 1:
                        for r in RORD:
                            nc.tensor.matmul(
                                ps[:, r * FD : (r + 1) * FD],
                                lhsT=w1r[:, 128 * r : 128 * (r + 1)],
                                rhs=px.h1[t - 1][:, :],
                                start=True,
                                stop=False,
                                tile_position=(0, 0),
                            )
                return ps

            def emit_step(px, layer, t, ps):
                if layer == 0:
                    if t >= 1:
                        for r in RORD:
                            nc.tensor.matmul(
                                ps[:, r * FD : (r + 1) * FD],
                                lhsT=wh0[:, 128 * r : 128 * (r + 1)],
                                rhs=px.h0[t - 1][:, :],
                                start=False,
                                stop=True,
                                tile_position=(0, 0),
                            )
                else:
                    for r in RORD:
                        nc.tensor.matmul(
                            ps[:, r * FD : (r + 1) * FD],
                            lhsT=w1i[:, 128 * r : 128 * (r + 1)],
                            rhs=px.h0[t][:, :],
                            start=(t == 0),
                            stop=True,
                            tile_position=(0, 0),
                        )

                cstate = px.c0 if layer == 0 else px.c1
                if layer == 0:
                    tg = gp.tile([P, FD], F32, name="tg")
                    nc.scalar.activation(tg[:], ps[:, 3 * FD : 4 * FD], AF.Tanh)
                    sifo = gp.tile([P, 3 * FD], F32, name="sifo")
                    nc.scalar.activation(sifo[:], ps[:, 0 : 3 * FD], AF.Sigmoid)
                    si = sifo[:, 0:FD]
                    sf = sifo[:, FD : 2 * FD]
                    so_ = sifo[:, 2 * FD : 3 * FD]
                else:
                    tg = gp.tile([P, FD], F32, name="tg")
                    nc.scalar.activation(
                        tg[:], ps[:, 3 * FD : 4 * FD], AF.Tanh, bias=bl1[:, 3:4]
                    )
                    si_t = gp.tile([P, FD], F32, name="si")
                    nc.scalar.activation(
                        si_t[:], ps[:, 0:FD], AF.Sigmoid, bias=bl1[:, 0:1]
                    )
                    si = si_t[:]
                    if t > 0:
                        sf_t = gp.tile([P, FD], F32, name="sf")
                        nc.scalar.activation(
                            sf_t[:], ps[:, FD : 2 * FD], AF.Sigmoid,
                            bias=bl1[:, 1:2],
                        )
                        sf = sf_t[:]
                    so_t = gp.tile([P, FD], BF16, name="so_")
                    nc.scalar.activation(
                        so_t[:], ps[:, 2 * FD : 3 * FD], AF.Sigmoid,
                        bias=bl1[:, 2:3],
                    )
                    so_ = so_t[:]

                if t == 0:
                    nc.vector.tensor_mul(cstate[:], si, tg[:])
                else:
                    t1 = gp.tile([P, FD], F32, name="t1")
                    nc.vector.tensor_mul(t1[:], si, tg[:])
                    t2 = gp.tile([P, FD], F32, name="t2")
                    nc.vector.tensor_mul(t2[:], sf, cstate[:])
                    nc.vector.tensor_add(cstate[:], t1[:], t2[:])

                tcx = gp.tile([P, FD], BF16, name="tcx")
                nc.scalar.activation(tcx[:], cstate[:], AF.Tanh)

                if layer == 0:
                    dst = px.h0[t]
                elif t < 3:
                    dst = px.h1[t]
                else:
                    dst = px.mlph
                nc.vector.tensor_mul(dst[:], so_, tcx[:])

            # ==== phase 1: LSTM stack as a sliding software pipeline with 4
            # pairs in flight (stagger 2 steps). Four independent recurrence
            # chains keep every engine fed, so cadence is engine-throughput
            # bound instead of chain-latency bound, and the PE stream stays
            # dense enough to hold the HAM clock gate at full speed.
            live = {}
            for s in range(2 * npair + 7):
                if s % 2 == 0 and s // 2 < npair:
                    live[s // 2] = open_pair(s // 2)
                for p in sorted(live):
                    step = s - 2 * p
                    if 0 <= step < 8:
                        layer, t = divmod(step, 4)
                        psx = emit_early(live[p], layer, t)
                        emit_step(live[p], layer, t, psx)
                for p in [p for p in live if s - 2 * p >= 7]:
                    persist.append((live[p].mlph, live[p].ft))
                    del live[p]

            # re-warm the PE across the phase transition
            wps2 = pp.tile([128, 2048], F32, name="ps")
            for k in range(32):
                bank = k % 4
                nc.tensor.matmul(
                    wps2[:, bank * FD : (bank + 1) * FD],
                    lhsT=wh0[:, 0:128],
                    rhs=wh0[:, 0:512],
                    start=(k < 4),
                    stop=(k >= 28),
                    tile_position=(0, 0),
                )
            wsb2 = const.tile([1, 4], F32, name="wsb2")
            nc.vector.tensor_copy(wsb2[:], wps2[0:1, 0:4])
            nc.sync.dma_start(warm_d[:, :], wsb2[:])

            # ======== phase 2: MLP + log_softmax, two pairs per block =======
            def emit_phase2(block, p0):
                nb = len(block)
                W = 1024 * nb
                ps = pp.tile([128, 2048], F32, name="ps")
                for i, (mlph, ft) in enumerate(block):
                    for half in (0, 1):
                        c = slice(1024 * i + 512 * half, 1024 * i + 512 * half + 512)
                        rows = slice(0, 64) if half == 0 else slice(64, 128)
                        frows = slice(0, 5) if half == 0 else slice(32, 37)
                        nc.tensor.matmul(
                            ps[0:30, c], lhsT=w1h[rows, :], rhs=mlph[rows, :],
                            start=True, stop=False,
                            tile_position=(0 if half == 0 else 64, 0),
                        )
                        nc.tensor.matmul(
                            ps[0:30, c], lhsT=w1f[frows, :], rhs=ft[frows, :],
                            start=False, stop=True,
                            tile_position=(0 if half == 0 else 32, 0),
                        )
                # stages live on disjoint partition rows (0/32/64/96) of the
                # same psum banks, so no bank write-after-read serialization
                m2s = p2.tile([43, 2048], BF16, name="m2s")
                nc.sync.dma_start(m2s[42:43, 0:W], onesbf[:, 0:W])
                m1s = p2.tile([30, 2048], BF16, name="m1s")
                nc.scalar.activation(
                    m1s[:, 0:W], ps[0:30, 0:W], AF.Relu, bias=bm1[:]
                )
                for j in range(2 * nb):
                    c = slice(512 * j, 512 * (j + 1))
                    nc.tensor.matmul(
                        ps[32:42, c], lhsT=wm2[:], rhs=m1s[:, c],
                        start=True, stop=True, tile_position=(0, 32),
                    )
                nc.scalar.activation(
                    m2s[32:42, 0:W], ps[32:42, 0:W], AF.Relu, bias=bm2[32:42, :]
                )
                for j in range(2 * nb):
                    c = slice(512 * j, 512 * (j + 1))
                    nc.tensor.matmul(
                        ps[64:68, c], lhsT=wm3[32:43, :], rhs=m2s[32:43, c],
                        start=True, stop=True, tile_position=(32, 64),
                    )
                es = p2.tile([68, 2048], F32, name="es", bufs=2)
                nc.scalar.activation(es[64:68, 0:W], ps[64:68, 0:W], AF.Exp)
                for j in range(2 * nb):
                    c = slice(512 * j, 512 * (j + 1))
                    nc.tensor.matmul(
                        ps[96:97, c], lhsT=ones4[64:68, :], rhs=es[64:68, c],
                        start=True, stop=True, tile_position=(64, 96),
                    )
                ls = p2.tile([97, 2048], F32, name="ls", bufs=2)
                nc.scalar.activation(ls[96:97, 0:W], ps[96:97, 0:W], AF.Ln)
                for j in range(2 * nb):
                    c = slice(512 * j, 512 * (j + 1))
                    nc.tensor.matmul(
                        ps[64:68, c], lhsT=nones[96:97, :], rhs=ls[96:97, c],
                        start=False, stop=True, tile_position=(96, 64),
                        skip_group_check=True,
                    )
                fo = p2.tile([68, 2048], F32, name="es", bufs=2)
                nc.vector.tensor_copy(fo[64:68, 0:W], ps[64:68, 0:W])
                nc.sync.dma_start(
                    out_d[:, 1024 * p0 : 1024 * p0 + W], fo[64:68, 0:W]
                )

            for b0 in range(0, npair, 2):
                emit_phase2(persist[b0 : b0 + 2], b0)

    return nc


def pack_weights(Wih0, Whh0, bih0, bhh0, Wih1, Whh1, bih1, bhh1,
                 W1, b1, W2, b2, W3, b3):
    bf = ml_dtypes.bfloat16
    b0 = bih0 + bhh0
    b1l = bih1 + bhh1
    wx6 = np.zeros((6, 512), np.float32)
    wh0 = np.zeros((128, 512), np.float32)
    w1i = np.zeros((128, 512), np.float32)
    w1r = np.zeros((128, 512), np.float32)
    bl1 = np.zeros((128, 4), np.float32)
    for r, sl in enumerate(GSLICE):
        cA = slice(128 * r, 128 * r + 64)
        cB = slice(128 * r + 64, 128 * r + 128)
        wx6[0:2, cA] = Wih0[sl].T
        wx6[2, cA] = b0[sl]
        wx6[3:5, cB] = Wih0[sl].T
        wx6[5, cB] = b0[sl]
        wh0[0:64, cA] = Whh0[sl].T
        wh0[64:128, cB] = Whh0[sl].T
        w1i[0:64, cA] = Wih1[sl].T
        w1i[64:128, cB] = Wih1[sl].T
        w1r[0:64, cA] = Whh1[sl].T
        w1r[64:128, cB] = Whh1[sl].T
        bl1[0:64, r] = b1l[sl]
        bl1[64:128, r] = b1l[sl]
    w1h = np.zeros((128, 30), np.float32)
    w1h[0:64] = W1[:, 0:64].T
    w1h[64:128] = W1[:, 0:64].T
    w1f = np.zeros((37, 30), np.float32)
    w1f[0:5] = W1[:, 64:69].T
    w1f[32:37] = W1[:, 64:69].T
    wm2 = np.ascontiguousarray(W2.T)
    wm3 = np.zeros((43, 4), np.float32)
    wm3[32:42] = W3.T
    wm3[42] = b3
    bm2 = np.zeros((42, 1), np.float32)
    bm2[32:42, 0] = b2
    negones = np.zeros((97, 4), np.float32)
    negones[96] = -1.0
    ones4 = np.zeros((68, 1), np.float32)
    ones4[64:68] = 1.0
    return {
        "wx6": wx6.astype(bf),
        "wh0d": wh0.astype(bf),
        "w1i": w1i.astype(bf),
        "w1r": w1r.astype(bf),
        "w1h": w1h.astype(bf),
        "w1f": w1f.astype(bf),
        "wm2": wm2.astype(bf),
        "wm3": wm3.astype(bf),
        "bl1": bl1,
        "bm1": np.ascontiguousarray(b1.reshape(30, 1), dtype=np.float32),
        "bm2": bm2,
        "negones": negones,
        "ones4": ones4,
    }


def pack_x(xs):
    """xs: [n, 13] f32 -> (xq [4, 6, n//2], fq [5, n]) bf16."""
    n = xs.shape[0]
    npair = n // (2 * FD)
    a = xs.reshape(npair, 2, FD, 13)
    A = a[:, 0].reshape(npair * FD, 13)
    Bv = a[:, 1].reshape(npair * FD, 13)
    xqv = np.zeros((4, 6, npair * FD), np.float32)
    for t in range(4):
        xqv[t, 0:2] = A[:, 2 * t : 2 * t + 2].T
        xqv[t, 2] = 1.0
        xqv[t, 3:5] = Bv[:, 2 * t : 2 * t + 2].T
        xqv[t, 5] = 1.0
    fqv = np.ascontiguousarray(xs[:, 8:13].T)
    return xqv.astype(ml_dtypes.bfloat16), fqv.astype(ml_dtypes.bfloat16)


_cached = {}


def run_cores(x, weights, trace=False):
    """x: [B_TOTAL, 13] f32. Returns (out [B_TOTAL, 4] f32, BassKernelResults)."""
    key = "prog"
    if key not in _cached:
        _cached[key] = build_program(NCHUNK)
    nc = _cached[key]
    in_maps = []
    for c in range(NCORES):
        xs = x[c * B_CORE : (c + 1) * B_CORE]
        m = dict(weights)
        m["xq"], m["fq"] = pack_x(xs)
        in_maps.append(m)
    res = run_bass_kernel_spmd(
        nc, in_maps, core_ids=list(range(NCORES)), trace=trace
    )
    outs = [res.results[c]["out"] for c in range(NCORES)]  # [4, 16384] each
    full = np.concatenate([o.T for o in outs], axis=0)     # [B_TOTAL, 4]
    return np.ascontiguousarray(full, dtype=np.float32), res


def kernel(x, Wih0, Whh0, bih0, bhh0, Wih1, Whh1, bih1, bhh1,
           W1, b1, W2, b2, W3, b3):
    args = [np.asarray(a, dtype=np.float32) for a in (
        Wih0, Whh0, bih0, bhh0, Wih1, Whh1, bih1, bhh1, W1, b1, W2, b2, W3, b3
    )]
    weights = pack_weights(*args)
    out, _ = run_cores(np.asarray(x, dtype=np.float32), weights)
    return out


# revision 29
# speedup vs baseline: 1.1606x; 1.1606x over previous
"""Trainium2 Bass kernel for nn_ActorNetwork (2-layer LSTM [T=4,H=64] + 3-layer
MLP + log_softmax over a batch of 131072 13-dim states).

Strategy: pure data parallel over 8 NeuronCores (16384 samples/core).
On-chip layout is feature-major (gate-major): gates/hidden units live on SBUF
partitions, samples on the free axis. Two 512-sample subtiles ("A" at
partitions 0:64, "B" at 64:128) are pair-packed so ACT/DVE ops run with all
128 partitions busy; gate matmuls use block-diagonal weights so one
K=128/M=128 matmul produces the gate for both subtiles. Two pairs are
emitted in lockstep so engines pipeline across them. Matmuls are bf16 with
fp32 PSUM accumulation; the cell state c and the softmax tail stay fp32.
All LSTM biases ride matmuls (ones-rows in the x blocks for layer 0, a
dedicated K=6 const matmul for layer 1) so the sigmoid over [I|F|O] is one
merged ACT op per step.
"""

import numpy as np
import ml_dtypes

import concourse.bass as bass
import concourse.mybir as mybir
from concourse.tile import TileContext
from concourse.bass_utils import run_bass_kernel_spmd
from concourse.vector_clock import ScopedClock
import concourse.tile as _tile_mod

BF16 = mybir.dt.bfloat16
F32 = mybir.dt.float32
AF = mybir.ActivationFunctionType

P = 128
FD = 512          # samples per subtile (= matmul free dim = one psum bank)
H = 64
NCORES = 8
B_TOTAL = 131072
B_CORE = B_TOTAL // NCORES          # 16384
NCHUNK = B_CORE // FD               # 32 subtiles/core
NPAIR = NCHUNK // 2                 # 16 pairs/core

# psum column region r -> PyTorch gate row range (PyTorch gate order i,f,g,o)
# regions ordered [i, f, o, g] so sigmoid covers a contiguous [I|F|O] block
GSLICE = [slice(0, 64), slice(64, 128), slice(192, 256), slice(128, 192)]
RORD = (3, 0, 1, 2)  # emit G first so tanh(G) unblocks the c-chain earliest

# ---------------------------------------------------------------------------
# walrus workaround: this toolchain rejects instructions carrying more than
# one sync wait; split excess waits onto same-engine nops inserted right
# before the offending instruction (identical engine-stream semantics).
_WAIT_LIMIT = 1


def _split_excess_waits(nc, limit=_WAIT_LIMIT):
    for f in nc.m.functions:
        for bb in f.blocks:
            snapshot = list(bb.instructions)
            out = []
            changed = False
            for inst in snapshot:
                si = getattr(inst, "sync_info", None)
                waits = list(si.on_wait) if si is not None else []
                if len(waits) > limit:
                    changed = True
                    extra, keep = waits[:-limit], waits[-limit:]
                    for w in extra:
                        b = nc.engines[inst.engine].nop(
                            nofuse=True, hint="wsplit"
                        )
                        ni = b.ins
                        cb = nc.cur_bb.bb
                        cb.instructions.remove(ni)
                        ni.sync_info = mybir.SyncInfo(
                            on_wait=[w], on_update=[]
                        )
                        out.append(ni)
                    inst.sync_info = mybir.SyncInfo(
                        on_wait=keep, on_update=list(si.on_update)
                    )
                out.append(inst)
            if changed:
                bb.instructions[:] = out


def _patched_drain_and_barrier(self, tick_clock, wait_clock):
    nc = self.nc
    drain_inst = nc.sync.drain()
    wait_clock.add_sem_waits(
        drain_inst.ins, ScopedClock({None: tick_clock.global_clock})
    )
    si = drain_inst.ins.sync_info
    waits = list(si.on_wait) if si is not None else []
    if len(waits) > _WAIT_LIMIT:
        drain_inst.ins.sync_info = mybir.SyncInfo(
            on_wait=waits[:_WAIT_LIMIT], on_update=list(si.on_update)
        )
        for k in range(_WAIT_LIMIT, len(waits), _WAIT_LIMIT):
            d2 = nc.sync.drain()
            d2.ins.sync_info = mybir.SyncInfo(
                on_wait=waits[k : k + _WAIT_LIMIT], on_update=[]
            )
    nc.all_engine_barrier()
    popped = nc._tile_sem_poison_stack.pop()
    assert popped is self._sem_poison
    nc.clear_and_free_semaphores(list(self.sems.allocated().values()))
    nc.all_engine_barrier()
    _split_excess_waits(nc)


_tile_mod.TileContext._drain_and_barrier = _patched_drain_and_barrier
# ---------------------------------------------------------------------------


class _PairCtx:
    __slots__ = ("idx", "xp6", "ft", "h0", "h1", "mlph", "c0", "c1")


def build_program(nchunk=NCHUNK):
    """Build the SPMD Bass program for one core processing nchunk*FD samples."""
    assert nchunk % 2 == 0
    npair = nchunk // 2
    ncols = nchunk * FD
    pcols = npair * FD

    nc = bass.Bass("TRN2", num_devices=NCORES)

    xq = nc.declare_dram_parameter("xq", [4, 6, pcols], BF16, isOutput=False)
    fq = nc.declare_dram_parameter("fq", [5, ncols], BF16, isOutput=False)
    wx6_d = nc.declare_dram_parameter("wx6", [6, 512], BF16, isOutput=False)
    wh0_d = nc.declare_dram_parameter("wh0d", [128, 512], BF16, isOutput=False)
    w1i_d = nc.declare_dram_parameter("w1i", [128, 512], BF16, isOutput=False)
    w1r_d = nc.declare_dram_parameter("w1r", [128, 512], BF16, isOutput=False)
    w1h_d = nc.declare_dram_parameter("w1h", [128, 30], BF16, isOutput=False)
    w1f_d = nc.declare_dram_parameter("w1f", [37, 30], BF16, isOutput=False)
    wm2_d = nc.declare_dram_parameter("wm2", [30, 10], BF16, isOutput=False)
    wm3_d = nc.declare_dram_parameter("wm3", [43, 4], BF16, isOutput=False)
    bl1_d = nc.declare_dram_parameter("bl1", [128, 4], F32, isOutput=False)
    bm1_d = nc.declare_dram_parameter("bm1", [30, 1], F32, isOutput=False)
    bm2_d = nc.declare_dram_parameter("bm2", [42, 1], F32, isOutput=False)
    no_d = nc.declare_dram_parameter("negones", [97, 4], F32, isOutput=False)
    o4_d = nc.declare_dram_parameter("ones4", [68, 1], F32, isOutput=False)
    out_d = nc.declare_dram_parameter("out", [4, ncols], F32, isOutput=True)
    warm_d = nc.declare_dram_parameter("warm", [1, 4], F32, isOutput=True)

    with TileContext(nc) as tc:
        with (
            tc.tile_pool(name="const", bufs=1) as const,
            tc.tile_pool(name="xpool", bufs=4) as xpool,
            tc.tile_pool(name="gp", bufs=3) as gp,
            tc.tile_pool(name="hp", bufs=4) as hp,
            tc.tile_pool(name="st", bufs=3) as st,
            tc.tile_pool(name="pers", bufs=npair) as pers,
            tc.tile_pool(name="pp", bufs=2, space="PSUM") as pp,
            tc.tile_pool(name="p2", bufs=2) as p2,
        ):
            # ---- constants -------------------------------------------------
            def cdma(name, dram, shape, dt=BF16):
                t = const.tile(shape, dt, name=name)
                nc.sync.dma_start(t[:], dram[:, :])
                return t

            wx6 = cdma("wx6", wx6_d, [6, 512])
            wh0 = cdma("wh0", wh0_d, [128, 512])
            w1i = cdma("w1i", w1i_d, [128, 512])
            w1r = cdma("w1r", w1r_d, [128, 512])
            w1h = cdma("w1h", w1h_d, [128, 30])
            w1f = cdma("w1f", w1f_d, [37, 30])
            wm2 = cdma("wm2", wm2_d, [30, 10])
            wm3 = cdma("wm3", wm3_d, [43, 4])
            bl1 = cdma("bl1", bl1_d, [128, 4], F32)
            bm1 = cdma("bm1", bm1_d, [30, 1], F32)
            bm2 = cdma("bm2", bm2_d, [42, 1], F32)
            nones = cdma("nones", no_d, [97, 4], F32)
            ones4 = cdma("ones4", o4_d, [68, 1], F32)
            onesbf = const.tile([1, 2048], BF16, name="onesbf")
            nc.vector.memset(onesbf[:], 1.0)

            # ---- PE warm-up: a dense burst of back-to-back matmuls so the
            # HAM clock gate reaches K=8/8 (2.4 GHz) before the real work.
            # Result is exported to a dummy output so nothing can DCE it.
            for blk in range(2):
                wps = pp.tile([128, 2048], F32, name="ps")
                for k in range(48):
                    bank = k % 4
                    nc.tensor.matmul(
                        wps[:, bank * FD : (bank + 1) * FD],
                        lhsT=wh0[:, 0:128],
                        rhs=wh0[:, 0:512],
                        start=(k < 4),
                        stop=(k >= 44),
                        tile_position=(0, 0),
                    )
                wsb = const.tile([1, 4], F32, name=f"wsb{blk}")
                nc.vector.tensor_copy(wsb[:], wps[0:1, 0:4])
                nc.sync.dma_start(warm_d[:, :], wsb[:])

            persist = []

            def open_pair(p):
                px = _PairCtx()
                px.idx = p
                pc = slice(p * FD, (p + 1) * FD)
                ca = slice(2 * p * FD, (2 * p + 1) * FD)
                cb = slice((2 * p + 1) * FD, (2 * p + 2) * FD)
                px.xp6 = []
                for t in range(4):
                    x6 = xpool.tile([6, FD], BF16, name=f"x6{t}")
                    nc.sync.dma_start(x6[:], xq[t, :, pc])
                    px.xp6.append(x6)
                ft = pers.tile([37, FD], BF16, name="ft")
                nc.sync.dma_start(ft[0:5, :], fq[:, ca])    # A feats
                nc.sync.dma_start(ft[32:37, :], fq[:, cb])  # B feats
                px.ft = ft
                px.mlph = pers.tile([P, FD], BF16, name="mlph")
                px.h0 = [hp.tile([P, FD], BF16, name=f"h0p{t}") for t in range(4)]
                px.h1 = [hp.tile([P, FD], BF16, name=f"h1p{t}") for t in range(3)]
                px.c0 = st.tile([P, FD], F32, name="c0")
                px.c1 = st.tile([P, FD], F32, name="c1")
                return px

            def emit_early(px, layer, t):
                ps = pp.tile([128, 2048], F32, name="ps")
                if layer == 0:
                    # x-projection (+bias ones-rows): depends only on the
                    # DMA'd x block and the psum slot -> runs well ahead of
                    # the recurrent chain and keeps PE busy.
                    for r in RORD:
                        nc.tensor.matmul(
                            ps[:, r * FD : (r + 1) * FD],
                            lhsT=wx6[:, 128 * r : 128 * (r + 1)],
                            rhs=px.xp6[t][:, :],
                            start=True,
                            stop=(t == 0),
                            tile_position=(0, 0),
                        )
                else:
                    if t >= 1:
                        for r in RORD:
                            nc.tensor.matmul(
                                ps[:, r * FD : (r + 1) * FD],
                                lhsT=w1r[:, 128 * r : 128 * (r + 1)],
                                rhs=px.h1[t - 1][:, :],
                                start=True,
                                stop=False,
                                tile_position=(0, 0),
                            )
                return ps

            def emit_step(px, layer, t, ps):
                if layer == 0:
                    if t >= 1:
                        for r in RORD:
                            nc.tensor.matmul(
                                ps[:, r * FD : (r + 1) * FD],
                                lhsT=wh0[:, 128 * r : 128 * (r + 1)],
                                rhs=px.h0[t - 1][:, :],
                                start=False,
                                stop=True,
                                tile_position=(0, 0),
                            )
                else:
                    for r in RORD:
                        nc.tensor.matmul(
                            ps[:, r * FD : (r + 1) * FD],
                            lhsT=w1i[:, 128 * r : 128 * (r + 1)],
                            rhs=px.h0[t][:, :],
                            start=(t == 0),
                            stop=True,
                            tile_position=(0, 0),
                        )

                cstate = px.c0 if layer == 0 else px.c1
                if layer == 0:
                    tg = gp.tile([P, FD], F32, name="tg")
                    nc.scalar.activation(tg[:], ps[:, 3 * FD : 4 * FD], AF.Tanh)
                    sifo = gp.tile([P, 3 * FD], F32, name="sifo")
                    nc.scalar.activation(sifo[:], ps[:, 0 : 3 * FD], AF.Sigmoid)
                    si = sifo[:, 0:FD]
                    sf = sifo[:, FD : 2 * FD]
                    so_ = sifo[:, 2 * FD : 3 * FD]
                else:
                    tg = gp.tile([P, FD], F32, name="tg")
                    nc.scalar.activation(
                        tg[:], ps[:, 3 * FD : 4 * FD], AF.Tanh, bias=bl1[:, 3:4]
                    )
                    si_t = gp.tile([P, FD], F32, name="si")
                    nc.scalar.activation(
                        si_t[:], ps[:, 0:FD], AF.Sigmoid, bias=bl1[:, 0:1]
                    )
                    si = si_t[:]
                    if t > 0:
                        sf_t = gp.tile([P, FD], F32, name="sf")
                        nc.scalar.activation(
                            sf_t[:], ps[:, FD : 2 * FD], AF.Sigmoid,
                            bias=bl1[:, 1:2],
                        )
                        sf = sf_t[:]
                    so_t = gp.tile([P, FD], BF16, name="so_")
                    nc.scalar.activation(
                        so_t[:], ps[:, 2 * FD : 3 * FD], AF.Sigmoid,
                        bias=bl1[:, 2:3],
                    )
                    so_ = so_t[:]

                if t == 0:
                    nc.vector.tensor_mul(cstate[:], si, tg[:])
                else:
                    t1 = gp.tile([P, FD], F32, name="t1")
                    nc.vector.tensor_mul(t1[:], si, tg[:])
                    t2 = gp.tile([P, FD], F32, name="t2")
                    nc.gpsimd.tensor_mul(t2[:], sf, cstate[:])
                    nc.vector.tensor_add(cstate[:], t1[:], t2[:])

                tcx = gp.tile([P, FD], BF16, name="tcx")
                nc.scalar.activation(tcx[:], cstate[:], AF.Tanh)

                if layer == 0:
                    dst = px.h0[t]
                elif t < 3:
                    dst = px.h1[t]
                else:
                    dst = px.mlph
                nc.vector.tensor_mul(dst[:], so_, tcx[:])

            # ==== phase 1: LSTM stack as a sliding software pipeline with 4
            # pairs in flight (stagger 2 steps). Four independent recurrence
            # chains keep every engine fed, so cadence is engine-throughput
            # bound instead of chain-latency bound, and the PE stream stays
            # dense enough to hold the HAM clock gate at full speed.
            live = {}
            for s in range(2 * npair + 7):
                if s % 2 == 0 and s // 2 < npair:
                    live[s // 2] = open_pair(s // 2)
                for p in sorted(live):
                    step = s - 2 * p
                    if 0 <= step < 8:
                        layer, t = divmod(step, 4)
                        psx = emit_early(live[p], layer, t)
                        emit_step(live[p], layer, t, psx)
                for p in [p for p in live if s - 2 * p >= 7]:
                    persist.append((live[p].mlph, live[p].ft))
                    del live[p]

            # ======== phase 2: MLP + log_softmax, two pairs per block =======
            def emit_phase2(block, p0):
                nb = len(block)
                W = 1024 * nb
                ps = pp.tile([128, 2048], F32, name="ps")
                for i, (mlph, ft) in enumerate(block):
                    for half in (0, 1):
                        c = slice(1024 * i + 512 * half, 1024 * i + 512 * half + 512)
                        rows = slice(0, 64) if half == 0 else slice(64, 128)
                        frows = slice(0, 5) if half == 0 else slice(32, 37)
                        nc.tensor.matmul(
                            ps[0:30, c], lhsT=w1h[rows, :], rhs=mlph[rows, :],
                            start=True, stop=False,
                            tile_position=(0 if half == 0 else 64, 0),
                        )
                        nc.tensor.matmul(
                            ps[0:30, c], lhsT=w1f[frows, :], rhs=ft[frows, :],
                            start=False, stop=True,
                            tile_position=(0 if half == 0 else 32, 0),
                        )
                # stages live on disjoint partition rows (0/32/64/96) of the
                # same psum banks, so no bank write-after-read serialization
                m2s = p2.tile([43, 2048], BF16, name="m2s")
                nc.sync.dma_start(m2s[42:43, 0:W], onesbf[:, 0:W])
                m1s = p2.tile([30, 2048], BF16, name="m1s")
                nc.scalar.activation(
                    m1s[:, 0:W], ps[0:30, 0:W], AF.Relu, bias=bm1[:]
                )
                for j in range(2 * nb):
                    c = slice(512 * j, 512 * (j + 1))
                    nc.tensor.matmul(
                        ps[32:42, c], lhsT=wm2[:], rhs=m1s[:, c],
                        start=True, stop=True, tile_position=(0, 32),
                    )
                nc.scalar.activation(
                    m2s[32:42, 0:W], ps[32:42, 0:W], AF.Relu, bias=bm2[32:42, :]
                )
                for j in range(2 * nb):
                    c = slice(512 * j, 512 * (j + 1))
                    nc.tensor.matmul(
                        ps[64:68, c], lhsT=wm3[32:43, :], rhs=m2s[32:43, c],
                        start=True, stop=True, tile_position=(32, 64),
                    )
                es = p2.tile([68, 2048], F32, name="es", bufs=2)
                nc.scalar.activation(es[64:68, 0:W], ps[64:68, 0:W], AF.Exp)
                for j in range(2 * nb):
                    c = slice(512 * j, 512 * (j + 1))
                    nc.tensor.matmul(
                        ps[96:97, c], lhsT=ones4[64:68, :], rhs=es[64:68, c],
                        start=True, stop=True, tile_position=(64, 96),
                    )
                ls = p2.tile([97, 2048], F32, name="ls", bufs=2)
                nc.scalar.activation(ls[96:97, 0:W], ps[96:97, 0:W], AF.Ln)
                for j in range(2 * nb):
                    c = slice(512 * j, 512 * (j + 1))
                    nc.tensor.matmul(
                        ps[64:68, c], lhsT=nones[96:97, :], rhs=ls[96:97, c],
                        start=False, stop=True, tile_position=(96, 64),
                        skip_group_check=True,
                    )
                fo = p2.tile([68, 2048], F32, name="es", bufs=2)
                nc.vector.tensor_copy(fo[64:68, 0:W], ps[64:68, 0:W])
                nc.sync.dma_start(
                    out_d[:, 1024 * p0 : 1024 * p0 + W], fo[64:68, 0:W]
                )

            for b0 in range(0, npair, 2):
                emit_phase2(persist[b0 : b0 + 2], b0)

    return nc


def pack_weights(Wih0, Whh0, bih0, bhh0, Wih1, Whh1, bih1, bhh1,
                 W1, b1, W2, b2, W3, b3):
    bf = ml_dtypes.bfloat16
    b0 = bih0 + bhh0
    b1l = bih1 + bhh1
    wx6 = np.zeros((6, 512), np.float32)
    wh0 = np.zeros((128, 512), np.float32)
    w1i = np.zeros((128, 512), np.float32)
    w1r = np.zeros((128, 512), np.float32)
    bl1 = np.zeros((128, 4), np.float32)
    for r, sl in enumerate(GSLICE):
        cA = slice(128 * r, 128 * r + 64)
        cB = slice(128 * r + 64, 128 * r + 128)
        wx6[0:2, cA] = Wih0[sl].T
        wx6[2, cA] = b0[sl]
        wx6[3:5, cB] = Wih0[sl].T
        wx6[5, cB] = b0[sl]
        wh0[0:64, cA] = Whh0[sl].T
        wh0[64:128, cB] = Whh0[sl].T
        w1i[0:64, cA] = Wih1[sl].T
        w1i[64:128, cB] = Wih1[sl].T
        w1r[0:64, cA] = Whh1[sl].T
        w1r[64:128, cB] = Whh1[sl].T
        bl1[0:64, r] = b1l[sl]
        bl1[64:128, r] = b1l[sl]
    w1h = np.zeros((128, 30), np.float32)
    w1h[0:64] = W1[:, 0:64].T
    w1h[64:128] = W1[:, 0:64].T
    w1f = np.zeros((37, 30), np.float32)
    w1f[0:5] = W1[:, 64:69].T
    w1f[32:37] = W1[:, 64:69].T
    wm2 = np.ascontiguousarray(W2.T)
    wm3 = np.zeros((43, 4), np.float32)
    wm3[32:42] = W3.T
    wm3[42] = b3
    bm2 = np.zeros((42, 1), np.float32)
    bm2[32:42, 0] = b2
    negones = np.zeros((97, 4), np.float32)
    negones[96] = -1.0
    ones4 = np.zeros((68, 1), np.float32)
    ones4[64:68] = 1.0
    return {
        "wx6": wx6.astype(bf),
        "wh0d": wh0.astype(bf),
        "w1i": w1i.astype(bf),
        "w1r": w1r.astype(bf),
        "w1h": w1h.astype(bf),
        "w1f": w1f.astype(bf),
        "wm2": wm2.astype(bf),
        "wm3": wm3.astype(bf),
        "bl1": bl1,
        "bm1": np.ascontiguousarray(b1.reshape(30, 1), dtype=np.float32),
        "bm2": bm2,
        "negones": negones,
        "ones4": ones4,
    }


def pack_x(xs):
    """xs: [n, 13] f32 -> (xq [4, 6, n//2], fq [5, n]) bf16."""
    n = xs.shape[0]
    npair = n // (2 * FD)
    a = xs.reshape(npair, 2, FD, 13)
    A = a[:, 0].reshape(npair * FD, 13)
    Bv = a[:, 1].reshape(npair * FD, 13)
    xqv = np.zeros((4, 6, npair * FD), np.float32)
    for t in range(4):
        xqv[t, 0:2] = A[:, 2 * t : 2 * t + 2].T
        xqv[t, 2] = 1.0
        xqv[t, 3:5] = Bv[:, 2 * t : 2 * t + 2].T
        xqv[t, 5] = 1.0
    fqv = np.ascontiguousarray(xs[:, 8:13].T)
    return xqv.astype(ml_dtypes.bfloat16), fqv.astype(ml_dtypes.bfloat16)


_cached = {}


def run_cores(x, weights, trace=False):
    """x: [B_TOTAL, 13] f32. Returns (out [B_TOTAL, 4] f32, BassKernelResults)."""
    key = "prog"
    if key not in _cached:
        _cached[key] = build_program(NCHUNK)
    nc = _cached[key]
    in_maps = []
    for c in range(NCORES):
        xs = x[c * B_CORE : (c + 1) * B_CORE]
        m = dict(weights)
        m["xq"], m["fq"] = pack_x(xs)
        in_maps.append(m)
    res = run_bass_kernel_spmd(
        nc, in_maps, core_ids=list(range(NCORES)), trace=trace
    )
    outs = [res.results[c]["out"] for c in range(NCORES)]  # [4, 16384] each
    full = np.concatenate([o.T for o in outs], axis=0)     # [B_TOTAL, 4]
    return np.ascontiguousarray(full, dtype=np.float32), res


def kernel(x, Wih0, Whh0, bih0, bhh0, Wih1, Whh1, bih1, bhh1,
           W1, b1, W2, b2, W3, b3):
    args = [np.asarray(a, dtype=np.float32) for a in (
        Wih0, Whh0, bih0, bhh0, Wih1, Whh1, bih1, bhh1, W1, b1, W2, b2, W3, b3
    )]
    weights = pack_weights(*args)
    out, _ = run_cores(np.asarray(x, dtype=np.float32), weights)
    return out
